# revision 4
# baseline (speedup 1.0000x reference)
"""SAGAN-style self-attention block on 8 Trainium2 NeuronCores (fp8 version).

Reference computation (per batch image, B=8, H=W=64, C=256, Cq=32):
    xf = x.reshape(N=4096, C)
    f = xf @ Wf + bf; g = xf @ Wg + bg; h = xf @ Wh + bh
    s = g @ f.T                  # [N, N]
    beta = softmax(s, axis=-1)
    o = beta @ h
    out = gamma * o + xf

Sharding: data-parallel over batch, one image per NeuronCore (8 cores),
no collectives.

Per-core kernel design:
  - Projections f/g/h run in bf16 (fp32 PSUM accumulation); f, g, h are
    stored in fp8e4m3 for the two big matmuls. g is pre-scaled by
    A_SCH = 4/ln2 so the fp8 exp bit-trick below needs no separate affine.
  - The score matmul s^T = f_aug^T @ g_aug and the output matmul o = e^T @ h
    both run in fp8 DoubleRow perf mode (two contraction slabs per
    instruction at 0.5 PE cycles per output column). The 33-row contraction
    (32 channels + 1 aug row) is split into 17+16 row slabs.
  - A per-query shift is folded into the score matmul via the augmented
    contraction row: f_aug row = 2.0 paired with g_aug row =
    (-A_SCH*M~_n + C_SCH)/2. Any consistent per-query shift divides out of
    softmax, so M~ only has to place each row's scores inside fp8e5m2's
    representable window. M~_n = max(samplemax_n, min(sig_n, samplemax_n
    + SIG_CAP)) + M_MARGIN, where samplemax is an exact row-max over the
    first 256 keys (computed by natural-orientation DR matmuls + DVE
    reduces) and sig_n = 4.078 * |g_n| * sqrt(tr(F^T F)/(32 N)) is an
    isotropic row-max estimate.
  - exp is a saturating approximation (cannot overflow -> no NaN/Inf):
      * ACT half: Sigmoid((s'' - C_SCH)/A_SCH) -> e5m2. Equals exp within
        ~2% for arguments <= -2.5 and saturates to 1 above.
      * DVE half: clamp(round(s''), 0, C_SCH) written as int8 and
        bit-cast to e5m2 (Schraudolph exponent trick, also capped at ~1).
    Each 512-query block runs 16 key-chunk-pair tiles; each tile's two
    512-col halves go to ACT and DVE concurrently from four single-bank
    PSUM score tiles (GPSIMD cannot read PSUM, so Pool only gets
    SBUF-side work: the residual add of the finalize, weight casts, and
    parts of the M~ chain).
  - o-matmuls trail the score/exp stream by LAG pair-tiles so the
    in-order PE never waits on a fresh exp result.
  - Row-sums come for free from a ones-column appended to h (h_aug[:, :C]
    already folds gamma and bias_h). The finalize adds EPS_ROWSUM before
    the reciprocal so fully-flushed rows degrade to the residual instead
    of NaN.
  - The residual add uses the original fp32 x, so for gamma == 0 the
    output is bit-exact x.
"""

import os
from contextlib import ExitStack

import numpy as np

import concourse.bass as bass
import concourse.tile as tile
from concourse import bacc, mybir
from concourse import bass_utils

N_CORES = 8
B, HH, WW, C = 8, 64, 64, 256
N = HH * WW        # 4096 pixels
CQ = C // 8        # 32
NCH = N // 128     # 32 chunks of 128 pixels
NB = N // 512      # 8 blocks of 512 score columns
HAUG = C + 1       # 257: h plus ones column

F32 = mybir.dt.float32
BF16 = mybir.dt.bfloat16
E4 = mybir.dt.float8e4
E5 = mybir.dt.float8e5
I8 = mybir.dt.int8
DR = mybir.MatmulPerfMode.DoubleRow
AF = mybir.ActivationFunctionType
ALU = mybir.AluOpType

LN2 = float(np.log(2.0))
A_SCH = 4.0 / LN2            # e5m2 exponent scale; g2 is pre-scaled by this
C_SCH = 60.0 - 0.25          # schraudolph constant (bias 15*4, tuned)
F_AUG = 2.0                  # f aug-row value (halves the g aug row range)
SIG_SCALE2 = (4.078 ** 2) / 32.0   # (sqrt(2 ln N))^2 / CQ, folded w/ tr(FTF)
SIG_CAP = 8.0                # cap sigma-estimate at samplemax + this
M_MARGIN = 3.0               # extra shift margin (score units)
EPS_ROWSUM = 1e-9
N_SAMP_CH = 2                # key chunks sampled for the row-max estimate

# exp engine per key-chunk-pair tile (16 per block): each tile's full
# [128,1024] score psum goes to ONE engine as a single instruction
# (fewer per-instruction fixed overheads). ACT is cheaper per element
# (0.83 vs 1.04 ns) so it gets a slightly larger share.
# A=ACT sigmoid, D=DVE schraudolph-clamp
PATT = ["A", "D"] * 8
PATT[15] = "A"  # 9 A / 7 D per block ~ 0.56 ACT share


def _bcast_ap(dram_ap, parts, free):
    return bass.AP(
        tensor=dram_ap.tensor,
        offset=dram_ap.offset,
        ap=[[0, parts], [1, free]],
    )


def _col_ap(dram_ap, n):
    """[n,1] column AP over a 1-D DRAM tensor."""
    return bass.AP(tensor=dram_ap.tensor, offset=dram_ap.offset,
                   ap=[[1, n], [0, 1]])


def _emit(ctx: ExitStack, tc: tile.TileContext, io: dict):
    nc = tc.nc
    xb, wf, wg, wh, bf, bg, bh, gamma, ob = (
        io["xb"], io["wf"], io["wg"], io["wh"],
        io["bf"], io["bg"], io["bh"], io["gamma"], io["ob"],
    )

    const = ctx.enter_context(tc.tile_pool(name="const", bufs=1))
    big = ctx.enter_context(tc.tile_pool(name="big", bufs=1))
    epool = ctx.enter_context(tc.tile_pool(name="epool", bufs=10))
    fin = ctx.enter_context(tc.tile_pool(name="fin", bufs=16))
    outp = ctx.enter_context(tc.tile_pool(name="outp", bufs=6))

    # ---- constants / weights ----------------------------------------------
    # preload the ACT sigmoid (+sqrt) tables while DMAs run
    junk = const.tile([128, 8], F32, tag="junk")
    nc.vector.memset(junk[:], 0.0)
    nc.scalar.activation(junk[:], junk[:], AF.Sigmoid)
    nc.scalar.activation(junk[:], junk[:], AF.Sqrt)
    nc.scalar.activation(junk[:], junk[:], AF.Square)

    # x prefetch (fp32), as in baseline
    xf_f32 = big.tile([128, NCH * C], F32, tag="xf_f32")
    xf_f32_3d = xf_f32[:].rearrange("p (i c) -> p i c", c=C)
    xb_3d = xb.rearrange("(i p) c -> p i c", p=128)
    nc.sync.dma_start(xf_f32_3d[:, 0:1, :], xb_3d[:, 0:1, :])
    nc.sync.dma_start(xf_f32_3d[:, 1:4, :], xb_3d[:, 1:4, :])
    nc.sync.dma_start(xf_f32_3d[:, 4:8, :], xb_3d[:, 4:8, :])

    # weights fp32 in -> bf16
    wf_f = const.tile([128, 2 * CQ], F32, tag="wf_f")
    wg_f = const.tile([128, 2 * CQ], F32, tag="wg_f")
    wh_f = const.tile([128, 2 * C], F32, tag="wh_f")
    for k in range(2):
        nc.sync.dma_start(wf_f[:, k * CQ:(k + 1) * CQ], wf[k * 128:(k + 1) * 128, :])
        nc.sync.dma_start(wg_f[:, k * CQ:(k + 1) * CQ], wg[k * 128:(k + 1) * 128, :])
        nc.sync.dma_start(wh_f[:, k * C:(k + 1) * C], wh[k * 128:(k + 1) * 128, :])
    wf_b = const.tile([128, 2 * CQ], BF16, tag="wf_b")
    wg_b = const.tile([128, 2 * CQ], BF16, tag="wg_b")
    wh_b = const.tile([128, 2 * C], BF16, tag="wh_b")
    nc.vector.tensor_copy(wf_b[:], wf_f[:])
    nc.vector.tensor_copy(wg_b[:], wg_f[:])
    nc.gpsimd.tensor_copy(wh_b[:], wh_f[:])

    # biases as [32,1] columns (channel on partition)
    bf_col = const.tile([CQ, 1], F32, tag="bf_col")
    nc.sync.dma_start(bf_col[:], _col_ap(bf, CQ))
    bg_col = const.tile([CQ, 1], F32, tag="bg_col")
    nc.sync.dma_start(bg_col[:], _col_ap(bg, CQ))
    bgs_col = const.tile([CQ, 1], F32, tag="bgs_col")
    nc.vector.tensor_scalar(bgs_col[:], bg_col[:], A_SCH, None, ALU.mult)
    bh_bc = const.tile([128, C], F32, tag="bh_bc")
    nc.sync.dma_start(bh_bc[:], _bcast_ap(bh, 128, C))
    gamma_bc = const.tile([128, 1], F32, tag="gamma_bc")
    nc.sync.dma_start(gamma_bc[:], _bcast_ap(gamma, 128, 1))
    bh_g = const.tile([128, C], F32, tag="bh_g")
    nc.vector.tensor_scalar_mul(bh_g[:], bh_bc[:], gamma_bc[:])
    bh_g2 = const.tile([128, 2 * C], F32, tag="bh_g2")
    nc.vector.tensor_copy(bh_g2[:, 0:C], bh_g[:])
    nc.gpsimd.tensor_copy(bh_g2[:, C:2 * C], bh_g[:])

    ident_f = const.tile([128, 128], F32, tag="ident_f")
    from concourse.masks import make_identity
    make_identity(nc, ident_f[:])
    ident_b = const.tile([128, 128], BF16, tag="ident_b")
    nc.gpsimd.tensor_copy(ident_b[:], ident_f[:])
    ident_b0 = const.tile([128, 128], BF16, tag="ident_b0")
    nc.gpsimd.tensor_copy(ident_b0[:], ident_f[:])

    # small consts
    two_e4 = const.tile([1, 128], E4, tag="two_e4")
    nc.vector.memset(two_e4[:], F_AUG)
    zero_e4 = const.tile([1, 128], E4, tag="zero_e4")
    nc.vector.memset(zero_e4[:], 0.0)
    sigb_bc = const.tile([128, 1], F32, tag="sigb_bc")
    nc.vector.memset(sigb_bc[:], -C_SCH / A_SCH)

    def bcast_sb(src_tile, parts, free):
        """AP replicating src_tile[0:1, ...] across partitions (and cols)."""
        ap = src_tile[0:1, 0:1]
        pstep = 1 if parts == 1 else 0
        if free <= 128:
            shape = [[pstep, parts], [1, free]]
        else:
            shape = [[pstep, parts], [0, free // 128], [1, 128]]
        return bass.AP(tensor=ap.tensor, offset=ap.offset, ap=shape)

    # ---- big SBUF tensors --------------------------------------------------
    xfT_bf = big.tile([128, 2 * N], BF16, tag="xfT_bf")          # [p,(k,n)]
    f2 = big.tile([17, NCH * 2 * 128], E4, tag="f2")             # [p,(m,sl,c)]
    g2 = big.tile([17, 2 * N], E4, tag="g2")                     # [p,(sl,n)]
    stage_f = big.tile([32, N], E4, tag="stage_f")               # parts 16:32
    stage_g = big.tile([32, N], E4, tag="stage_g")
    h_aug = big.tile([128, NCH * HAUG], E4, tag="h_aug")
    g_nat = big.tile([128, NCH * CQ], BF16, tag="g_nat")
    gsqn = big.tile([128, NCH * CQ], BF16, tag="gsqn")
    fsq_s = big.tile([32, 512], BF16, tag="fsq_s")               # scratch
    facc = big.tile([32, 8], F32, tag="facc")
    m_col = big.tile([128, NCH], BF16, tag="m_col")
    gn_col = big.tile([128, NCH], F32, tag="gn_col")
    mneg = big.tile([128, NCH], BF16, tag="mneg")
    mstage = big.tile([32, 128], E4, tag="mstage")
    trv8 = big.tile([1, 8], F32, tag="trv8")
    trv_bc = big.tile([128, 1], F32, tag="trv_bc")
    scale_bc = big.tile([128, 1], F32, tag="scale_bc")
    sig_col = big.tile([128, NCH], F32, tag="sig_col")
    t2_col = big.tile([128, NCH], F32, tag="t2_col")
    t1_col = big.tile([128, NCH], F32, tag="t1_col")
    t3_col = big.tile([128, NCH], F32, tag="t3_col")

    f2_4d = f2[:].rearrange("p (m sl c) -> p m sl c", sl=2, c=128)
    g2_3d = g2[:].rearrange("p (sl n) -> p sl n", sl=2)
    xfT_3d = xfT_bf[:].rearrange("p (k n) -> p k n", k=2)
    h_aug_3d = h_aug[:].rearrange("p (m c) -> p m c", c=HAUG)
    g_nat_3d = g_nat[:].rearrange("p (m c) -> p m c", c=CQ)

    # round-robin engine dispatch for copy-out distribution. GPSIMD cannot
    # read PSUM, so drains rotate between ACT and DVE only.
    _engines = [nc.scalar, nc.vector]
    _rr = [0]

    def rr_copy(out, in_):
        e = _engines[_rr[0] % 2]
        _rr[0] += 1
        if e is nc.scalar:
            nc.scalar.copy(out, in_)
        else:
            e.tensor_copy(out, in_)

    def rr_add(out, in_, bias_ap):
        e = _engines[_rr[0] % 2]
        _rr[0] += 1
        if e is nc.scalar:
            nc.scalar.activation(out, in_, AF.Identity, bias=bias_ap)
        else:
            e.tensor_scalar(out, in_, bias_ap, None, ALU.add)

    def rr_add_scale(out, in_, bias_ap, scaled_bias_ap, scale):
        """out = (in_ + bias) * scale."""
        e = _engines[_rr[0] % 2]
        _rr[0] += 1
        if e is nc.scalar:
            nc.scalar.activation(out, in_, AF.Identity, scale=scale,
                                 bias=scaled_bias_ap)
        else:
            e.tensor_scalar(out, in_, bias_ap, scale, ALU.add, ALU.mult)

    # ---- prologue: transposes, projections, sample-max (interleaved) ------
    # aug rows first (consts only): f2 slab0 row16 = F_AUG, slab1 row16 = 0;
    # g2 row16 = 0 in both slabs (slab0 is read as 0 by the sample-max
    # matmuls, then overwritten with the -M~ row)
    nc.sync.dma_start(f2_4d[16:17, :, 0, :], bcast_sb(two_e4, 1, N))
    nc.sync.dma_start(f2_4d[16:17, :, 1, :], bcast_sb(zero_e4, 1, N))
    nc.sync.dma_start(g2_3d[16:17, 1, :], bcast_sb(zero_e4, 1, N))
    nc.sync.dma_start(g2_3d[16:17, 0, :], bcast_sb(zero_e4, 1, N))

    with tc.tile_pool(name="ps_t", bufs=2, space="PSUM") as ps_t, \
         tc.tile_pool(name="ps_w", bufs=4, space="PSUM") as ps_w, \
         tc.tile_pool(name="ps_sub", bufs=2, space="PSUM") as ps_sub:

        def emit_mops(c0, c1):
            sl = slice(c0, c1)
            nc.scalar.activation(sig_col[:, sl], gn_col[:, sl], AF.Sqrt,
                                 scale=scale_bc[:])
            nc.gpsimd.tensor_scalar(t1_col[:, sl], m_col[:, sl],
                                    SIG_CAP * A_SCH, None, ALU.add)
            nc.vector.tensor_tensor(t2_col[:, sl], sig_col[:, sl],
                                    t1_col[:, sl], op=ALU.min)
            nc.vector.tensor_tensor(t3_col[:, sl], m_col[:, sl],
                                    t2_col[:, sl], op=ALU.max)
            nc.gpsimd.tensor_scalar(mneg[:, sl], t3_col[:, sl],
                                    -1.0 / F_AUG,
                                    (C_SCH - M_MARGIN * A_SCH) / F_AUG,
                                    ALU.mult, ALU.add)

        def emit_mfold(c0, c1):
            w = c1 - c0
            ps_mt = ps_w.tile([w, 128], BF16, tag="w", name=f"psmt{c0}")
            nc.tensor.transpose(ps_mt[:], mneg[:, c0:c1], ident_b[:])
            mst = fin.tile([32, 128], E4, tag="mst", name=f"mst{c0}")
            nc.scalar.copy(mst[0:w, :], ps_mt[:])
            nc.sync.dma_start(g2_3d[16:17, 0, c0 * 128:c1 * 128], mst[0:w, :])

        for mt in range(8):
            pf = mt + 2
            if pf < 8:
                nc.sync.dma_start(xf_f32_3d[:, pf * 4:(pf + 1) * 4, :],
                                  xb_3d[:, pf * 4:(pf + 1) * 4, :])
            # PE-transpose 8 [128,128] fp32 blocks -> xfT bf16
            for k in range(2):
                tp = ps_t.tile([128, 512], F32, tag="tp", name=f"tp{mt}_{k}")
                for idx, i in enumerate(range(mt * 4, mt * 4 + 4)):
                    nc.tensor.transpose(
                        tp[:, idx * 128:(idx + 1) * 128],
                        xf_f32[:, i * C + k * 128: i * C + k * 128 + 128],
                        ident_f[:],
                    )
                nc.scalar.copy(xfT_3d[:, k, mt * 512:(mt + 1) * 512], tp[:])

            # f^T, g^T ([32, 512] psum, channels on partitions)
            ps_f = ps_w.tile([32, 512], F32, tag="w", name=f"psf{mt}")
            ps_g = ps_w.tile([32, 512], F32, tag="w", name=f"psg{mt}")
            for k in range(2):
                nc.tensor.matmul(ps_f[:], lhsT=wf_b[:, k * CQ:(k + 1) * CQ],
                                 rhs=xfT_3d[:, k, mt * 512:(mt + 1) * 512],
                                 start=(k == 0), stop=(k == 1))
            for k in range(2):
                nc.tensor.matmul(ps_g[:], lhsT=wg_b[:, k * CQ:(k + 1) * CQ],
                                 rhs=xfT_3d[:, k, mt * 512:(mt + 1) * 512],
                                 start=(k == 0), stop=(k == 1))

            # copy-outs with bias into full-width staging (both slabs in
            # one instruction); g is pre-scaled by A_SCH so exp becomes a
            # single clamp. DMAs below remap the halves into slab layout.
            nc.scalar.activation(stage_f[:, mt * 512:(mt + 1) * 512],
                                 ps_f[:], AF.Identity, bias=bf_col[:])
            nc.vector.tensor_scalar(stage_g[:, mt * 512:(mt + 1) * 512],
                                    ps_g[:], bg_col[:], A_SCH,
                                    ALU.add, ALU.mult)

            # |f|^2 accumulation for tr(F^T F) (group 0 sample is enough
            # for this global scale estimate)
            if mt == 0:
                nc.scalar.activation(fsq_s[:], ps_f[:], AF.Square,
                                     accum_out=facc[:, 0:1])

            # slab remap DMAs for this group (stage -> f2/g2 layouts)
            nc.sync.dma_start(f2_4d[0:16, mt * 4:(mt + 1) * 4, 0, :],
                              stage_f[0:16, mt * 512:(mt + 1) * 512])
            nc.sync.dma_start(f2_4d[0:16, mt * 4:(mt + 1) * 4, 1, :],
                              stage_f[16:32, mt * 512:(mt + 1) * 512])
            nc.sync.dma_start(g2_3d[0:16, 0, mt * 512:(mt + 1) * 512],
                              stage_g[0:16, mt * 512:(mt + 1) * 512])
            nc.sync.dma_start(g2_3d[0:16, 1, mt * 512:(mt + 1) * 512],
                              stage_g[16:32, mt * 512:(mt + 1) * 512])

            # g natural (for |g_n|^2): one [128, 128] psum per group
            ps_gn = ps_w.tile([128, 4 * CQ], F32, tag="w", name=f"psgn{mt}")
            for j in range(4):
                m = mt * 4 + j
                for k in range(2):
                    nc.tensor.matmul(ps_gn[:, j * CQ:(j + 1) * CQ],
                                     lhsT=xfT_3d[:, k, m * 128:(m + 1) * 128],
                                     rhs=wg_b[:, k * CQ:(k + 1) * CQ],
                                     start=(k == 0), stop=(k == 1))
            nc.vector.tensor_copy(g_nat_3d[:, mt * 4:(mt + 1) * 4, :], ps_gn[:])

            # h (bf16) + copy-out folding gamma & bias -> fp8 h_aug
            for j2 in range(2):
                ps_h = ps_w.tile([128, 2 * C], F32, tag="w",
                                 name=f"psh{mt}_{j2}")
                for jj in range(2):
                    m = mt * 4 + 2 * j2 + jj
                    for k in range(2):
                        nc.tensor.matmul(
                            ps_h[:, jj * C:(jj + 1) * C],
                            lhsT=xfT_3d[:, k, m * 128:(m + 1) * 128],
                            rhs=wh_b[:, k * C:(k + 1) * C],
                            start=(k == 0), stop=(k == 1))
                m0 = mt * 4 + 2 * j2
                nc.scalar.activation(h_aug_3d[:, m0:m0 + 2, 0:C], ps_h[:],
                                     AF.Identity, scale=gamma_bc[:])
                nc.gpsimd.tensor_tensor(h_aug_3d[:, m0:m0 + 2, 0:C],
                                        h_aug_3d[:, m0:m0 + 2, 0:C],
                                        bh_g2[:], op=ALU.add)

            # |g_n|^2 incrementally for this group (Pool square + DVE
            # inner-axis reduce) so only the tail remains after group 7
            gsl = slice(mt * 4 * CQ, (mt + 1) * 4 * CQ)
            nc.gpsimd.tensor_tensor(gsqn[:, gsl], g_nat[:, gsl],
                                    g_nat[:, gsl], op=ALU.mult)
            gsq_g = gsqn[:, gsl].rearrange("p (m c) -> p m c", c=CQ)
            nc.vector.tensor_reduce(gn_col[:, mt * 4:(mt + 1) * 4], gsq_g,
                                    mybir.AxisListType.X, ALU.add)

            # tr(F^T F) estimate from the first 7 groups: start the DRAM
            # broadcast roundtrip early so it is off the critical path
            if mt == 0:
                facc_b = big.tile([32, 8], BF16, tag="facc_b")
                nc.gpsimd.tensor_copy(facc_b[:, 0:1], facc[:, 0:1])
                ones32 = const.tile([32, 1], BF16, tag="ones32")
                nc.vector.memset(ones32[:], 1.0)
                ps_tr = ps_w.tile([1, 8], F32, tag="w", name="ps_tr")
                nc.tensor.matmul(ps_tr[:, 0:1], lhsT=ones32[:],
                                 rhs=facc_b[:, 0:1], start=True, stop=True)
                trv1 = big.tile([1, 1], F32, tag="trv1")
                nc.scalar.copy(trv1[:], ps_tr[:, 0:1])
                scr_trv = nc.dram_tensor("scr_trv", [1], F32,
                                         kind="Internal").ap()
                nc.sync.dma_start(scr_trv, trv1[:])
                nc.sync.dma_start(trv_bc[:], _bcast_ap(scr_trv, 128, 1))
                nc.vector.tensor_scalar(scale_bc[:], trv_bc[:],
                                        8.0 * SIG_SCALE2 * A_SCH
                                        * A_SCH / N, None, ALU.mult)

            if mt == 7:
                emit_mfold(0, 28)

            # sample-max for this group's 4 query chunks (needs f2 chunks
            # 0..1 => group 0, and this group's g2 columns); two query
            # chunks share one reduce instruction
            for j in range(2):
                qc0 = mt * 4 + 2 * j
                ps_ss = ps_sub.tile([128, 512], F32, tag="ss",
                                    name=f"ss{qc0}")
                for jj in range(2):
                    for m in range(N_SAMP_CH):
                        nc.tensor.matmul(
                            ps_ss[:, jj * 256 + m * 128:
                                  jj * 256 + (m + 1) * 128],
                            lhsT=g2_3d[:, :, (qc0 + jj) * 128:
                                       (qc0 + jj + 1) * 128],
                            rhs=f2_4d[:, m, :, :],
                            start=True, stop=True, perf_mode=DR)
                red_in = ps_ss[:].rearrange("p (a c) -> p a c", c=256)
                nc.vector.tensor_reduce(m_col[:, qc0:qc0 + 2], red_in,
                                        mybir.AxisListType.X, ALU.max)
            if mt == 6:
                emit_mops(0, 28)

    # h_aug ones column
    nc.vector.memset(h_aug_3d[:, :, C:C + 1], 1.0)

    # ---- prologue tail: last 4 columns of the -M~ aug row ------------------
    with tc.tile_pool(name="ps_m", bufs=1, space="PSUM") as ps_m:
        emit_mops(28, 32)
        ps_mt2 = ps_m.tile([4, 128], BF16, tag="ps_mt2")
        nc.tensor.transpose(ps_mt2[:], mneg[:, 28:32], ident_b[:])
        nc.scalar.copy(mstage[0:4, :], ps_mt2[:])
        nc.sync.dma_start(g2_3d[16:17, 0, 28 * 128:32 * 128], mstage[0:4, :])

    # ---- main attention loop ----------------------------------------------
    ps_s = ctx.enter_context(tc.tile_pool(name="ps_s", bufs=2, space="PSUM"))
    ps_o = ctx.enter_context(tc.tile_pool(name="ps_o", bufs=4, space="PSUM"))
    ob_3d = ob.rearrange("(k p) c -> p k c", p=128)

    LAG = 4  # o-matmuls trail the s/exp stream by this many pair-tiles
    o_tiles: dict = {}
    e_tiles: dict = {}

    def emit_o(p):
        nb2, t2 = p // 16, p % 16
        o_ps = o_tiles[nb2]
        e_3d = e_tiles.pop(p)[:].rearrange("p (sl n) -> p sl n", sl=2)
        for q in range(4):
            nc.tensor.matmul(
                o_ps[q][:], lhsT=e_3d[:, :, q * 128:(q + 1) * 128],
                rhs=h_aug_3d[:, 2 * t2:2 * t2 + 2, :],
                start=(t2 == 0), stop=(t2 == 15), perf_mode=DR)
        if t2 == 15:
            finalize(nb2)

    def finalize(nb2):
        o_ps = o_tiles.pop(nb2)
        res4 = outp.tile([128, 4 * C], F32, tag="res4", name=f"res4_{nb2}")
        for q in range(4):
            gch = nb2 * 4 + q
            rs = fin.tile([128, 1], F32, tag="rs", name=f"rs{nb2}_{q}")
            nc.vector.tensor_scalar(rs[:], o_ps[q][:, C:C + 1], EPS_ROWSUM,
                                    None, ALU.add)
            recip = fin.tile([128, 1], F32, tag="recip", name=f"rc{nb2}_{q}")
            nc.vector.reciprocal_approx_fast(recip[:], rs[:])
            res_sc = fin.tile([128, C], F32, tag="res_sc",
                              name=f"rsc{nb2}_{q}")
            nc.scalar.activation(res_sc[:], o_ps[q][:, 0:C], AF.Identity,
                                 scale=recip[:])
            nc.gpsimd.tensor_tensor(
                res4[:, q * C:(q + 1) * C], res_sc[:],
                xf_f32[:, gch * C:(gch + 1) * C], op=ALU.add)
        nc.sync.dma_start(
            ob_3d[:, nb2 * 4:(nb2 + 1) * 4, :],
            res4[:].rearrange("p (k c) -> p k c", c=C))

    for p in range(NB * 16):
        nb, t = p // 16, p % 16
        if t == 0:
            o_tiles[nb] = [
                ps_o.tile([128, HAUG], F32, tag="o", name=f"o_ps{nb}_{q}")
                for q in range(4)]
        g2_blk = g2_3d[:, :, nb * 512:(nb + 1) * 512]
        e_t = epool.tile([128, 1024], E5, tag="e", name=f"e{nb}_{t}")
        e_tiles[p] = e_t
        s_ps = ps_s.tile([128, 1024], F32, tag="s", name=f"s{nb}_{t}")
        for hh in range(2):
            nc.tensor.matmul(s_ps[:, hh * 512:(hh + 1) * 512],
                             lhsT=f2_4d[:, 2 * t + hh, :, :],
                             rhs=g2_blk, start=True, stop=True, perf_mode=DR)
        if PATT[t] == "A":
            # s'' holds A_SCH*(s - M~) + C_SCH; undo for the sigmoid
            nc.scalar.activation(e_t[:], s_ps[:], AF.Sigmoid,
                                 scale=1.0 / A_SCH, bias=sigb_bc[:])
        else:
            nc.vector.tensor_scalar(e_t[:].bitcast(I8), s_ps[:],
                                    0.0, C_SCH, ALU.max, ALU.min)
        if p >= LAG:
            emit_o(p - LAG)
    for p in range(NB * 16 - LAG, NB * 16):
        emit_o(p)


_CACHE: dict = {}


def build():
    if "nc" in _CACHE:
        return _CACHE["nc"]
    nc = bacc.Bacc("TRN2", target_bir_lowering=False, debug=False,
                   num_devices=N_CORES)
    io = {
        "xb": nc.dram_tensor("xb", [N, C], F32, kind="ExternalInput").ap(),
        "wf": nc.dram_tensor("wf", [C, CQ], F32, kind="ExternalInput").ap(),
        "wg": nc.dram_tensor("wg", [C, CQ], F32, kind="ExternalInput").ap(),
        "wh": nc.dram_tensor("wh", [C, C], F32, kind="ExternalInput").ap(),
        "bf": nc.dram_tensor("bf", [CQ], F32, kind="ExternalInput").ap(),
        "bg": nc.dram_tensor("bg", [CQ], F32, kind="ExternalInput").ap(),
        "bh": nc.dram_tensor("bh", [C], F32, kind="ExternalInput").ap(),
        "gamma": nc.dram_tensor("gamma", [1], F32, kind="ExternalInput").ap(),
        "ob": nc.dram_tensor("ob", [N, C], F32, kind="ExternalOutput").ap(),
    }
    with tile.TileContext(nc) as tc:
        with ExitStack() as ctx:
            _emit(ctx, tc, io)
    nc.compile()
    _CACHE["nc"] = nc
    return nc


def kernel(x, kernel_f, kernel_g, kernel_h, bias_f, bias_g, bias_h, gamma):
    x = np.asarray(x, dtype=np.float32)
    wf = np.ascontiguousarray(np.asarray(kernel_f, dtype=np.float32))
    wg = np.ascontiguousarray(np.asarray(kernel_g, dtype=np.float32))
    wh = np.ascontiguousarray(np.asarray(kernel_h, dtype=np.float32))
    bf = np.ascontiguousarray(np.asarray(bias_f, dtype=np.float32))
    bg = np.ascontiguousarray(np.asarray(bias_g, dtype=np.float32))
    bh = np.ascontiguousarray(np.asarray(bias_h, dtype=np.float32))
    gm = np.ascontiguousarray(np.asarray(gamma, dtype=np.float32).reshape(1))

    per_core = {
        "xb": [np.ascontiguousarray(x[b].reshape(N, C)) for b in range(N_CORES)],
        "wf": [wf] * N_CORES, "wg": [wg] * N_CORES, "wh": [wh] * N_CORES,
        "bf": [bf] * N_CORES, "bg": [bg] * N_CORES, "bh": [bh] * N_CORES,
        "gamma": [gm] * N_CORES,
    }
    nc = build()
    in_maps = [{nm: per_core[nm][b] for nm in per_core} for b in range(N_CORES)]
    try:
        res = bass_utils.run_bass_kernel_spmd(
            nc, in_maps, core_ids=list(range(N_CORES)))
    except ModuleNotFoundError:
        os.environ["BASS_NEVER_TRACE"] = "1"
        res = bass_utils.run_bass_kernel_spmd(
            nc, in_maps, core_ids=list(range(N_CORES)))
    out = np.stack([res.results[b]["ob"] for b in range(N_CORES)], axis=0)
    return out.reshape(B, HH, WW, C).astype(np.float32)


if __name__ == "__main__":
    rng = np.random.default_rng(0)
    x = rng.standard_normal((B, HH, WW, C)).astype(np.float32)
    lim = np.sqrt(6.0 / (C + CQ))
    out = kernel(
        x,
        rng.uniform(-lim, lim, (C, CQ)).astype(np.float32),
        rng.uniform(-lim, lim, (C, CQ)).astype(np.float32),
        rng.uniform(-lim, lim, (C, C)).astype(np.float32),
        np.zeros(CQ, np.float32), np.zeros(CQ, np.float32),
        np.zeros(C, np.float32), np.zeros(1, np.float32),
    )
    print(out.shape, out.dtype)



# revision 8
# speedup vs baseline: 1.1782x; 1.1782x over previous
"""SAGAN-style self-attention block on 8 Trainium2 NeuronCores (fp8 version).

Reference computation (per batch image, B=8, H=W=64, C=256, Cq=32):
    xf = x.reshape(N=4096, C)
    f = xf @ Wf + bf; g = xf @ Wg + bg; h = xf @ Wh + bh
    s = g @ f.T                  # [N, N]
    beta = softmax(s, axis=-1)
    o = beta @ h
    out = gamma * o + xf

Sharding: data-parallel over batch, one image per NeuronCore (8 cores),
no collectives.

Per-core kernel design:
  - Projections f/g/h run in bf16 (fp32 PSUM accumulation); f, g, h are
    stored in fp8e4m3 for the two big matmuls. g is pre-scaled by
    A_SCH = 4/ln2 so the fp8 exp bit-trick below needs no separate affine.
  - The score matmul s^T = f_aug^T @ g_aug and the output matmul o = e^T @ h
    both run in fp8 DoubleRow perf mode (two contraction slabs per
    instruction at 0.5 PE cycles per output column). The 33-row contraction
    (32 channels + 1 aug row) is split into 17+16 row slabs.
  - A per-query shift is folded into the score matmul via the augmented
    contraction row: f_aug row = 2.0 paired with g_aug row =
    (-A_SCH*M~_n + C_SCH)/2. Any consistent per-query shift divides out of
    softmax, so M~ only has to place each row's scores inside fp8e5m2's
    representable window. M~_n = max(samplemax_n, min(sig_n, samplemax_n
    + SIG_CAP)) + M_MARGIN, where samplemax is an exact row-max over the
    first 256 keys (computed by natural-orientation DR matmuls + DVE
    reduces) and sig_n = 4.078 * |g_n| * sqrt(tr(F^T F)/(32 N)) is an
    isotropic row-max estimate.
  - exp is a saturating approximation (cannot overflow -> no NaN/Inf):
      * ACT half: Sigmoid((s'' - C_SCH)/A_SCH) -> e5m2. Equals exp within
        ~2% for arguments <= -2.5 and saturates to 1 above.
      * DVE half: clamp(round(s''), 0, C_SCH) written as int8 and
        bit-cast to e5m2 (Schraudolph exponent trick, also capped at ~1).
    Each 512-query block runs 16 key-chunk-pair tiles; each tile's two
    512-col halves go to ACT and DVE concurrently from four single-bank
    PSUM score tiles (GPSIMD cannot read PSUM, so Pool only gets
    SBUF-side work: the residual add of the finalize, weight casts, and
    parts of the M~ chain).
  - o-matmuls trail the score/exp stream by LAG pair-tiles so the
    in-order PE never waits on a fresh exp result.
  - Row-sums come for free from a ones-column appended to h (h_aug[:, :C]
    already folds gamma and bias_h). The finalize adds EPS_ROWSUM before
    the reciprocal so fully-flushed rows degrade to the residual instead
    of NaN.
  - The residual add uses the original fp32 x, so for gamma == 0 the
    output is bit-exact x.
"""

import os
from contextlib import ExitStack

import numpy as np

import concourse.bass as bass
import concourse.tile as tile
from concourse import bacc, mybir
from concourse import bass_utils

N_CORES = 8
B, HH, WW, C = 8, 64, 64, 256
N = HH * WW        # 4096 pixels
CQ = C // 8        # 32
NCH = N // 128     # 32 chunks of 128 pixels
NB = N // 512      # 8 blocks of 512 score columns
HAUG = C + 1       # 257: h plus ones column

F32 = mybir.dt.float32
BF16 = mybir.dt.bfloat16
E4 = mybir.dt.float8e4
E5 = mybir.dt.float8e5
I8 = mybir.dt.int8
DR = mybir.MatmulPerfMode.DoubleRow
AF = mybir.ActivationFunctionType
ALU = mybir.AluOpType

LN2 = float(np.log(2.0))
A_SCH = 4.0 / LN2            # e5m2 exponent scale; g2 is pre-scaled by this
C_SCH = 60.0 - 0.25          # schraudolph constant (bias 15*4, tuned)
F_AUG = 2.0                  # f aug-row value (halves the g aug row range)
SIG_SCALE2 = (4.078 ** 2) / 32.0   # (sqrt(2 ln N))^2 / CQ, folded w/ tr(FTF)
SIG_CAP = 8.0                # cap sigma-estimate at samplemax + this
M_MARGIN = 3.0               # extra shift margin (score units)
EPS_ROWSUM = 1e-9
N_SAMP_CH = 2                # key chunks sampled for the row-max estimate

# exp engine pair per key-chunk-pair tile (16 per block): each tile's two
# 512-col halves go to two DIFFERENT engines so they run concurrently and
# the s-psum frees after ~one half-instruction latency.
# A=ACT sigmoid, D=DVE schraudolph-clamp
PATT = ["AD"] * 16


def _bcast_ap(dram_ap, parts, free):
    return bass.AP(
        tensor=dram_ap.tensor,
        offset=dram_ap.offset,
        ap=[[0, parts], [1, free]],
    )


def _col_ap(dram_ap, n):
    """[n,1] column AP over a 1-D DRAM tensor."""
    return bass.AP(tensor=dram_ap.tensor, offset=dram_ap.offset,
                   ap=[[1, n], [0, 1]])


def _emit(ctx: ExitStack, tc: tile.TileContext, io: dict):
    nc = tc.nc
    xb, wf, wg, wh, bf, bg, bh, gamma, ob = (
        io["xb"], io["wf"], io["wg"], io["wh"],
        io["bf"], io["bg"], io["bh"], io["gamma"], io["ob"],
    )

    const = ctx.enter_context(tc.tile_pool(name="const", bufs=1))
    big = ctx.enter_context(tc.tile_pool(name="big", bufs=1))
    epool = ctx.enter_context(tc.tile_pool(name="epool", bufs=10))
    fin = ctx.enter_context(tc.tile_pool(name="fin", bufs=16))
    outp = ctx.enter_context(tc.tile_pool(name="outp", bufs=6))

    # ---- constants / weights ----------------------------------------------
    # preload the ACT sigmoid (+sqrt) tables while DMAs run
    junk = const.tile([128, 8], F32, tag="junk")
    nc.vector.memset(junk[:], 0.0)
    nc.scalar.activation(junk[:], junk[:], AF.Sigmoid)
    nc.scalar.activation(junk[:], junk[:], AF.Sqrt)
    nc.scalar.activation(junk[:], junk[:], AF.Square)

    # x prefetch (fp32), as in baseline
    xf_f32 = big.tile([128, NCH * C], F32, tag="xf_f32")
    xf_f32_3d = xf_f32[:].rearrange("p (i c) -> p i c", c=C)
    xb_3d = xb.rearrange("(i p) c -> p i c", p=128)
    nc.sync.dma_start(xf_f32_3d[:, 0:1, :], xb_3d[:, 0:1, :])
    nc.sync.dma_start(xf_f32_3d[:, 1:4, :], xb_3d[:, 1:4, :])
    nc.sync.dma_start(xf_f32_3d[:, 4:8, :], xb_3d[:, 4:8, :])

    # weights fp32 in -> bf16
    wf_f = const.tile([128, 2 * CQ], F32, tag="wf_f")
    wg_f = const.tile([128, 2 * CQ], F32, tag="wg_f")
    wh_f = const.tile([128, 2 * C], F32, tag="wh_f")
    for k in range(2):
        nc.sync.dma_start(wf_f[:, k * CQ:(k + 1) * CQ], wf[k * 128:(k + 1) * 128, :])
        nc.sync.dma_start(wg_f[:, k * CQ:(k + 1) * CQ], wg[k * 128:(k + 1) * 128, :])
        nc.sync.dma_start(wh_f[:, k * C:(k + 1) * C], wh[k * 128:(k + 1) * 128, :])
    wf_b = const.tile([128, 2 * CQ], BF16, tag="wf_b")
    wg_b = const.tile([128, 2 * CQ], BF16, tag="wg_b")
    wh_b = const.tile([128, 2 * C], BF16, tag="wh_b")
    nc.vector.tensor_copy(wf_b[:], wf_f[:])
    nc.vector.tensor_copy(wg_b[:], wg_f[:])
    nc.gpsimd.tensor_copy(wh_b[:], wh_f[:])

    # biases as [32,1] columns (channel on partition)
    bf_col = const.tile([CQ, 1], F32, tag="bf_col")
    nc.sync.dma_start(bf_col[:], _col_ap(bf, CQ))
    bg_col = const.tile([CQ, 1], F32, tag="bg_col")
    nc.sync.dma_start(bg_col[:], _col_ap(bg, CQ))
    bgs_col = const.tile([CQ, 1], F32, tag="bgs_col")
    nc.vector.tensor_scalar(bgs_col[:], bg_col[:], A_SCH, None, ALU.mult)
    bh_bc = const.tile([128, C], F32, tag="bh_bc")
    nc.sync.dma_start(bh_bc[:], _bcast_ap(bh, 128, C))
    gamma_bc = const.tile([128, 1], F32, tag="gamma_bc")
    nc.sync.dma_start(gamma_bc[:], _bcast_ap(gamma, 128, 1))
    bh_g = const.tile([128, C], F32, tag="bh_g")
    nc.vector.tensor_scalar_mul(bh_g[:], bh_bc[:], gamma_bc[:])
    bh_g2 = const.tile([128, 2 * C], F32, tag="bh_g2")
    nc.vector.tensor_copy(bh_g2[:, 0:C], bh_g[:])
    nc.gpsimd.tensor_copy(bh_g2[:, C:2 * C], bh_g[:])

    ident_f = const.tile([128, 128], F32, tag="ident_f")
    from concourse.masks import make_identity
    make_identity(nc, ident_f[:])
    ident_b = const.tile([128, 128], BF16, tag="ident_b")
    nc.gpsimd.tensor_copy(ident_b[:], ident_f[:])
    ident_b0 = const.tile([128, 128], BF16, tag="ident_b0")
    nc.gpsimd.tensor_copy(ident_b0[:], ident_f[:])

    # small consts
    two_e4 = const.tile([1, 128], E4, tag="two_e4")
    nc.vector.memset(two_e4[:], F_AUG)
    zero_e4 = const.tile([1, 128], E4, tag="zero_e4")
    nc.vector.memset(zero_e4[:], 0.0)
    sigb_bc = const.tile([128, 1], F32, tag="sigb_bc")
    nc.vector.memset(sigb_bc[:], -C_SCH / A_SCH)

    def bcast_sb(src_tile, parts, free):
        """AP replicating src_tile[0:1, ...] across partitions (and cols)."""
        ap = src_tile[0:1, 0:1]
        pstep = 1 if parts == 1 else 0
        if free <= 128:
            shape = [[pstep, parts], [1, free]]
        else:
            shape = [[pstep, parts], [0, free // 128], [1, 128]]
        return bass.AP(tensor=ap.tensor, offset=ap.offset, ap=shape)

    # ---- big SBUF tensors --------------------------------------------------
    xfT_bf = big.tile([128, 2 * N], BF16, tag="xfT_bf")          # [p,(k,n)]
    f2 = big.tile([17, NCH * 2 * 128], E4, tag="f2")             # [p,(m,sl,c)]
    g2 = big.tile([17, 2 * N], E4, tag="g2")                     # [p,(sl,n)]
    stage_f = big.tile([32, N], E4, tag="stage_f")               # parts 16:32
    stage_g = big.tile([32, N], E4, tag="stage_g")
    h_aug = big.tile([128, NCH * HAUG], E4, tag="h_aug")
    g_nat = big.tile([128, NCH * CQ], BF16, tag="g_nat")
    gsqn = big.tile([128, NCH * CQ], BF16, tag="gsqn")
    fsq_s = big.tile([32, 512], BF16, tag="fsq_s")               # scratch
    facc = big.tile([32, 8], F32, tag="facc")
    m_col = big.tile([128, NCH], BF16, tag="m_col")
    gn_col = big.tile([128, NCH], F32, tag="gn_col")
    mneg = big.tile([128, NCH], BF16, tag="mneg")
    mstage = big.tile([32, 128], E4, tag="mstage")
    trv8 = big.tile([1, 8], F32, tag="trv8")
    trv_bc = big.tile([128, 1], F32, tag="trv_bc")
    scale_bc = big.tile([128, 1], F32, tag="scale_bc")
    sig_col = big.tile([128, NCH], F32, tag="sig_col")
    t2_col = big.tile([128, NCH], F32, tag="t2_col")
    t1_col = big.tile([128, NCH], F32, tag="t1_col")
    t3_col = big.tile([128, NCH], F32, tag="t3_col")

    f2_4d = f2[:].rearrange("p (m sl c) -> p m sl c", sl=2, c=128)
    g2_3d = g2[:].rearrange("p (sl n) -> p sl n", sl=2)
    xfT_3d = xfT_bf[:].rearrange("p (k n) -> p k n", k=2)
    h_aug_3d = h_aug[:].rearrange("p (m c) -> p m c", c=HAUG)
    g_nat_3d = g_nat[:].rearrange("p (m c) -> p m c", c=CQ)

    # round-robin engine dispatch for copy-out distribution. GPSIMD cannot
    # read PSUM, so drains rotate between ACT and DVE only.
    _engines = [nc.scalar, nc.vector]
    _rr = [0]

    def rr_copy(out, in_):
        e = _engines[_rr[0] % 2]
        _rr[0] += 1
        if e is nc.scalar:
            nc.scalar.copy(out, in_)
        else:
            e.tensor_copy(out, in_)

    def rr_add(out, in_, bias_ap):
        e = _engines[_rr[0] % 2]
        _rr[0] += 1
        if e is nc.scalar:
            nc.scalar.activation(out, in_, AF.Identity, bias=bias_ap)
        else:
            e.tensor_scalar(out, in_, bias_ap, None, ALU.add)

    def rr_add_scale(out, in_, bias_ap, scaled_bias_ap, scale):
        """out = (in_ + bias) * scale."""
        e = _engines[_rr[0] % 2]
        _rr[0] += 1
        if e is nc.scalar:
            nc.scalar.activation(out, in_, AF.Identity, scale=scale,
                                 bias=scaled_bias_ap)
        else:
            e.tensor_scalar(out, in_, bias_ap, scale, ALU.add, ALU.mult)

    # ---- prologue: transposes, projections, sample-max (interleaved) ------
    # aug rows first (consts only): f2 slab0 row16 = F_AUG, slab1 row16 = 0;
    # g2 row16 = 0 in both slabs (slab0 is read as 0 by the sample-max
    # matmuls, then overwritten with the -M~ row)
    nc.sync.dma_start(f2_4d[16:17, :, 0, :], bcast_sb(two_e4, 1, N))
    nc.sync.dma_start(f2_4d[16:17, :, 1, :], bcast_sb(zero_e4, 1, N))
    nc.sync.dma_start(g2_3d[16:17, 1, :], bcast_sb(zero_e4, 1, N))
    nc.sync.dma_start(g2_3d[16:17, 0, :], bcast_sb(zero_e4, 1, N))

    with tc.tile_pool(name="ps_t", bufs=2, space="PSUM") as ps_t, \
         tc.tile_pool(name="ps_w", bufs=4, space="PSUM") as ps_w, \
         tc.tile_pool(name="ps_sub", bufs=2, space="PSUM") as ps_sub:

        def emit_mops(c0, c1):
            sl = slice(c0, c1)
            nc.scalar.activation(sig_col[:, sl], gn_col[:, sl], AF.Sqrt,
                                 scale=scale_bc[:])
            nc.gpsimd.tensor_scalar(t1_col[:, sl], m_col[:, sl],
                                    SIG_CAP * A_SCH, None, ALU.add)
            nc.vector.tensor_tensor(t2_col[:, sl], sig_col[:, sl],
                                    t1_col[:, sl], op=ALU.min)
            nc.vector.tensor_tensor(t3_col[:, sl], m_col[:, sl],
                                    t2_col[:, sl], op=ALU.max)
            nc.gpsimd.tensor_scalar(mneg[:, sl], t3_col[:, sl],
                                    -1.0 / F_AUG,
                                    (C_SCH - M_MARGIN * A_SCH) / F_AUG,
                                    ALU.mult, ALU.add)

        def emit_mfold(c0, c1):
            w = c1 - c0
            ps_mt = ps_w.tile([w, 128], BF16, tag="w", name=f"psmt{c0}")
            nc.tensor.transpose(ps_mt[:], mneg[:, c0:c1], ident_b[:])
            mst = fin.tile([32, 128], E4, tag="mst", name=f"mst{c0}")
            nc.scalar.copy(mst[0:w, :], ps_mt[:])
            nc.sync.dma_start(g2_3d[16:17, 0, c0 * 128:c1 * 128], mst[0:w, :])

        for mt in range(8):
            pf = mt + 2
            if pf < 8:
                nc.sync.dma_start(xf_f32_3d[:, pf * 4:(pf + 1) * 4, :],
                                  xb_3d[:, pf * 4:(pf + 1) * 4, :])
            # PE-transpose 8 [128,128] fp32 blocks -> xfT bf16
            for k in range(2):
                tp = ps_t.tile([128, 512], F32, tag="tp", name=f"tp{mt}_{k}")
                for idx, i in enumerate(range(mt * 4, mt * 4 + 4)):
                    nc.tensor.transpose(
                        tp[:, idx * 128:(idx + 1) * 128],
                        xf_f32[:, i * C + k * 128: i * C + k * 128 + 128],
                        ident_f[:],
                    )
                nc.scalar.copy(xfT_3d[:, k, mt * 512:(mt + 1) * 512], tp[:])

            # f^T, g^T ([32, 512] psum, channels on partitions)
            ps_f = ps_w.tile([32, 512], F32, tag="w", name=f"psf{mt}")
            ps_g = ps_w.tile([32, 512], F32, tag="w", name=f"psg{mt}")
            for k in range(2):
                nc.tensor.matmul(ps_f[:], lhsT=wf_b[:, k * CQ:(k + 1) * CQ],
                                 rhs=xfT_3d[:, k, mt * 512:(mt + 1) * 512],
                                 start=(k == 0), stop=(k == 1))
            for k in range(2):
                nc.tensor.matmul(ps_g[:], lhsT=wg_b[:, k * CQ:(k + 1) * CQ],
                                 rhs=xfT_3d[:, k, mt * 512:(mt + 1) * 512],
                                 start=(k == 0), stop=(k == 1))

            # copy-outs with bias into full-width staging (both slabs in
            # one instruction); g is pre-scaled by A_SCH so exp becomes a
            # single clamp. DMAs below remap the halves into slab layout.
            nc.scalar.activation(stage_f[:, mt * 512:(mt + 1) * 512],
                                 ps_f[:], AF.Identity, bias=bf_col[:])
            nc.vector.tensor_scalar(stage_g[:, mt * 512:(mt + 1) * 512],
                                    ps_g[:], bg_col[:], A_SCH,
                                    ALU.add, ALU.mult)

            # |f|^2 accumulation for tr(F^T F) (group 0 sample is enough
            # for this global scale estimate)
            if mt == 0:
                nc.scalar.activation(fsq_s[:], ps_f[:], AF.Square,
                                     accum_out=facc[:, 0:1])

            # slab remap DMAs for this group (stage -> f2/g2 layouts)
            nc.sync.dma_start(f2_4d[0:16, mt * 4:(mt + 1) * 4, 0, :],
                              stage_f[0:16, mt * 512:(mt + 1) * 512])
            nc.sync.dma_start(f2_4d[0:16, mt * 4:(mt + 1) * 4, 1, :],
                              stage_f[16:32, mt * 512:(mt + 1) * 512])
            nc.sync.dma_start(g2_3d[0:16, 0, mt * 512:(mt + 1) * 512],
                              stage_g[0:16, mt * 512:(mt + 1) * 512])
            nc.sync.dma_start(g2_3d[0:16, 1, mt * 512:(mt + 1) * 512],
                              stage_g[16:32, mt * 512:(mt + 1) * 512])

            # g natural (for |g_n|^2): one [128, 128] psum per group
            ps_gn = ps_w.tile([128, 4 * CQ], F32, tag="w", name=f"psgn{mt}")
            for j in range(4):
                m = mt * 4 + j
                for k in range(2):
                    nc.tensor.matmul(ps_gn[:, j * CQ:(j + 1) * CQ],
                                     lhsT=xfT_3d[:, k, m * 128:(m + 1) * 128],
                                     rhs=wg_b[:, k * CQ:(k + 1) * CQ],
                                     start=(k == 0), stop=(k == 1))
            nc.vector.tensor_copy(g_nat_3d[:, mt * 4:(mt + 1) * 4, :], ps_gn[:])

            # h (bf16) + copy-out folding gamma & bias -> fp8 h_aug
            for j2 in range(2):
                ps_h = ps_w.tile([128, 2 * C], F32, tag="w",
                                 name=f"psh{mt}_{j2}")
                for jj in range(2):
                    m = mt * 4 + 2 * j2 + jj
                    for k in range(2):
                        nc.tensor.matmul(
                            ps_h[:, jj * C:(jj + 1) * C],
                            lhsT=xfT_3d[:, k, m * 128:(m + 1) * 128],
                            rhs=wh_b[:, k * C:(k + 1) * C],
                            start=(k == 0), stop=(k == 1))
                m0 = mt * 4 + 2 * j2
                nc.scalar.activation(h_aug_3d[:, m0:m0 + 2, 0:C], ps_h[:],
                                     AF.Identity, scale=gamma_bc[:])
                nc.gpsimd.tensor_tensor(h_aug_3d[:, m0:m0 + 2, 0:C],
                                        h_aug_3d[:, m0:m0 + 2, 0:C],
                                        bh_g2[:], op=ALU.add)

            # |g_n|^2 incrementally for this group (Pool square + DVE
            # inner-axis reduce) so only the tail remains after group 7
            gsl = slice(mt * 4 * CQ, (mt + 1) * 4 * CQ)
            nc.gpsimd.tensor_tensor(gsqn[:, gsl], g_nat[:, gsl],
                                    g_nat[:, gsl], op=ALU.mult)
            gsq_g = gsqn[:, gsl].rearrange("p (m c) -> p m c", c=CQ)
            nc.vector.tensor_reduce(gn_col[:, mt * 4:(mt + 1) * 4], gsq_g,
                                    mybir.AxisListType.X, ALU.add)

            # tr(F^T F) estimate from the first 7 groups: start the DRAM
            # broadcast roundtrip early so it is off the critical path
            if mt == 0:
                facc_b = big.tile([32, 8], BF16, tag="facc_b")
                nc.gpsimd.tensor_copy(facc_b[:, 0:1], facc[:, 0:1])
                ones32 = const.tile([32, 1], BF16, tag="ones32")
                nc.vector.memset(ones32[:], 1.0)
                ps_tr = ps_w.tile([1, 8], F32, tag="w", name="ps_tr")
                nc.tensor.matmul(ps_tr[:, 0:1], lhsT=ones32[:],
                                 rhs=facc_b[:, 0:1], start=True, stop=True)
                trv1 = big.tile([1, 1], F32, tag="trv1")
                nc.scalar.copy(trv1[:], ps_tr[:, 0:1])
                scr_trv = nc.dram_tensor("scr_trv", [1], F32,
                                         kind="Internal").ap()
                nc.sync.dma_start(scr_trv, trv1[:])
                nc.sync.dma_start(trv_bc[:], _bcast_ap(scr_trv, 128, 1))
                nc.vector.tensor_scalar(scale_bc[:], trv_bc[:],
                                        8.0 * SIG_SCALE2 * A_SCH
                                        * A_SCH / N, None, ALU.mult)

            if mt == 7:
                emit_mfold(0, 28)

            # sample-max for this group's 4 query chunks (needs f2 chunks
            # 0..1 => group 0, and this group's g2 columns); two query
            # chunks share one reduce instruction
            for j in range(2):
                qc0 = mt * 4 + 2 * j
                ps_ss = ps_sub.tile([128, 512], F32, tag="ss",
                                    name=f"ss{qc0}")
                for jj in range(2):
                    for m in range(N_SAMP_CH):
                        nc.tensor.matmul(
                            ps_ss[:, jj * 256 + m * 128:
                                  jj * 256 + (m + 1) * 128],
                            lhsT=g2_3d[:, :, (qc0 + jj) * 128:
                                       (qc0 + jj + 1) * 128],
                            rhs=f2_4d[:, m, :, :],
                            start=True, stop=True, perf_mode=DR)
                red_in = ps_ss[:].rearrange("p (a c) -> p a c", c=256)
                nc.vector.tensor_reduce(m_col[:, qc0:qc0 + 2], red_in,
                                        mybir.AxisListType.X, ALU.max)
            if mt == 6:
                emit_mops(0, 28)

    # h_aug ones column
    nc.vector.memset(h_aug_3d[:, :, C:C + 1], 1.0)

    # ---- prologue tail: last 4 columns of the -M~ aug row ------------------
    with tc.tile_pool(name="ps_m", bufs=1, space="PSUM") as ps_m:
        emit_mops(28, 32)
        ps_mt2 = ps_m.tile([4, 128], BF16, tag="ps_mt2")
        nc.tensor.transpose(ps_mt2[:], mneg[:, 28:32], ident_b[:])
        nc.scalar.copy(mstage[0:4, :], ps_mt2[:])
        nc.sync.dma_start(g2_3d[16:17, 0, 28 * 128:32 * 128], mstage[0:4, :])

    # ---- main attention loop ----------------------------------------------
    ps_s = ctx.enter_context(tc.tile_pool(name="ps_s", bufs=4, space="PSUM"))
    ps_o = ctx.enter_context(tc.tile_pool(name="ps_o", bufs=4, space="PSUM"))
    ob_3d = ob.rearrange("(k p) c -> p k c", p=128)

    LAG = 4  # o-matmuls trail the s/exp stream by this many pair-tiles
    o_tiles: dict = {}
    e_tiles: dict = {}

    def emit_o(p):
        nb2, t2 = p // 16, p % 16
        o_ps = o_tiles[nb2]
        e_3d = e_tiles.pop(p)[:].rearrange("p (sl n) -> p sl n", sl=2)
        for q in range(4):
            nc.tensor.matmul(
                o_ps[q][:], lhsT=e_3d[:, :, q * 128:(q + 1) * 128],
                rhs=h_aug_3d[:, 2 * t2:2 * t2 + 2, :],
                start=(t2 == 0), stop=(t2 == 15), perf_mode=DR)
        if t2 == 15:
            finalize(nb2)

    def finalize(nb2):
        o_ps = o_tiles.pop(nb2)
        res4 = outp.tile([128, 4 * C], F32, tag="res4", name=f"res4_{nb2}")
        for q in range(4):
            gch = nb2 * 4 + q
            rs = fin.tile([128, 1], F32, tag="rs", name=f"rs{nb2}_{q}")
            nc.vector.tensor_scalar(rs[:], o_ps[q][:, C:C + 1], EPS_ROWSUM,
                                    None, ALU.add)
            recip = fin.tile([128, 1], F32, tag="recip", name=f"rc{nb2}_{q}")
            nc.vector.reciprocal_approx_fast(recip[:], rs[:])
            res_sc = fin.tile([128, C], F32, tag="res_sc",
                              name=f"rsc{nb2}_{q}")
            nc.scalar.activation(res_sc[:], o_ps[q][:, 0:C], AF.Identity,
                                 scale=recip[:])
            nc.gpsimd.tensor_tensor(
                res4[:, q * C:(q + 1) * C], res_sc[:],
                xf_f32[:, gch * C:(gch + 1) * C], op=ALU.add)
        nc.sync.dma_start(
            ob_3d[:, nb2 * 4:(nb2 + 1) * 4, :],
            res4[:].rearrange("p (k c) -> p k c", c=C))

    for p in range(NB * 16):
        nb, t = p // 16, p % 16
        if t == 0:
            o_tiles[nb] = [
                ps_o.tile([128, HAUG], F32, tag="o", name=f"o_ps{nb}_{q}")
                for q in range(4)]
        g2_blk = g2_3d[:, :, nb * 512:(nb + 1) * 512]
        e_t = epool.tile([128, 1024], E5, tag="e", name=f"e{nb}_{t}")
        e_tiles[p] = e_t
        for hh, eng_c in enumerate(PATT[t]):
            s_ps = ps_s.tile([128, 512], F32, tag="s", name=f"s{nb}_{t}_{hh}")
            nc.tensor.matmul(s_ps[:], lhsT=f2_4d[:, 2 * t + hh, :, :],
                             rhs=g2_blk, start=True, stop=True, perf_mode=DR)
            sl = slice(hh * 512, (hh + 1) * 512)
            if eng_c == "A":
                # s'' holds A_SCH*(s - M~) + C_SCH; undo for the sigmoid
                nc.scalar.activation(e_t[:, sl], s_ps[:], AF.Sigmoid,
                                     scale=1.0 / A_SCH, bias=sigb_bc[:])
            else:
                eng = nc.vector if eng_c == "D" else nc.gpsimd
                eng.tensor_scalar(e_t[:, sl].bitcast(I8), s_ps[:],
                                  0.0, C_SCH, ALU.max, ALU.min)
        if p >= LAG:
            emit_o(p - LAG)
    for p in range(NB * 16 - LAG, NB * 16):
        emit_o(p)


_CACHE: dict = {}


def build():
    if "nc" in _CACHE:
        return _CACHE["nc"]
    nc = bacc.Bacc("TRN2", target_bir_lowering=False, debug=False,
                   num_devices=N_CORES)
    io = {
        "xb": nc.dram_tensor("xb", [N, C], F32, kind="ExternalInput").ap(),
        "wf": nc.dram_tensor("wf", [C, CQ], F32, kind="ExternalInput").ap(),
        "wg": nc.dram_tensor("wg", [C, CQ], F32, kind="ExternalInput").ap(),
        "wh": nc.dram_tensor("wh", [C, C], F32, kind="ExternalInput").ap(),
        "bf": nc.dram_tensor("bf", [CQ], F32, kind="ExternalInput").ap(),
        "bg": nc.dram_tensor("bg", [CQ], F32, kind="ExternalInput").ap(),
        "bh": nc.dram_tensor("bh", [C], F32, kind="ExternalInput").ap(),
        "gamma": nc.dram_tensor("gamma", [1], F32, kind="ExternalInput").ap(),
        "ob": nc.dram_tensor("ob", [N, C], F32, kind="ExternalOutput").ap(),
    }
    with tile.TileContext(nc) as tc:
        with ExitStack() as ctx:
            _emit(ctx, tc, io)
    nc.compile()
    _CACHE["nc"] = nc
    return nc


def kernel(x, kernel_f, kernel_g, kernel_h, bias_f, bias_g, bias_h, gamma):
    x = np.asarray(x, dtype=np.float32)
    wf = np.ascontiguousarray(np.asarray(kernel_f, dtype=np.float32))
    wg = np.ascontiguousarray(np.asarray(kernel_g, dtype=np.float32))
    wh = np.ascontiguousarray(np.asarray(kernel_h, dtype=np.float32))
    bf = np.ascontiguousarray(np.asarray(bias_f, dtype=np.float32))
    bg = np.ascontiguousarray(np.asarray(bias_g, dtype=np.float32))
    bh = np.ascontiguousarray(np.asarray(bias_h, dtype=np.float32))
    gm = np.ascontiguousarray(np.asarray(gamma, dtype=np.float32).reshape(1))

    per_core = {
        "xb": [np.ascontiguousarray(x[b].reshape(N, C)) for b in range(N_CORES)],
        "wf": [wf] * N_CORES, "wg": [wg] * N_CORES, "wh": [wh] * N_CORES,
        "bf": [bf] * N_CORES, "bg": [bg] * N_CORES, "bh": [bh] * N_CORES,
        "gamma": [gm] * N_CORES,
    }
    nc = build()
    in_maps = [{nm: per_core[nm][b] for nm in per_core} for b in range(N_CORES)]
    try:
        res = bass_utils.run_bass_kernel_spmd(
            nc, in_maps, core_ids=list(range(N_CORES)))
    except ModuleNotFoundError:
        os.environ["BASS_NEVER_TRACE"] = "1"
        res = bass_utils.run_bass_kernel_spmd(
            nc, in_maps, core_ids=list(range(N_CORES)))
    out = np.stack([res.results[b]["ob"] for b in range(N_CORES)], axis=0)
    return out.reshape(B, HH, WW, C).astype(np.float32)


if __name__ == "__main__":
    rng = np.random.default_rng(0)
    x = rng.standard_normal((B, HH, WW, C)).astype(np.float32)
    lim = np.sqrt(6.0 / (C + CQ))
    out = kernel(
        x,
        rng.uniform(-lim, lim, (C, CQ)).astype(np.float32),
        rng.uniform(-lim, lim, (C, CQ)).astype(np.float32),
        rng.uniform(-lim, lim, (C, C)).astype(np.float32),
        np.zeros(CQ, np.float32), np.zeros(CQ, np.float32),
        np.zeros(C, np.float32), np.zeros(1, np.float32),
    )
    print(out.shape, out.dtype)

